# revision 4
# baseline (speedup 1.0000x reference)
"""Trainium2 Bass kernel for nn_Decoder_3289944948995 (GNN message-passing decoder).

Reference computation (per edge e):
    z   = concat(z_drug[row[e]], z_reaction[col[e]])          # [2H] = [1024]
    h   = relu(W1 @ z + b1)                                   # [512]
    out = W2 @ h + b2                                         # scalar

Algebraic restructure: W1 @ concat(zd, zr) = W1d @ zd + W1r @ zr, so
    A = z_drug     @ W1[:, :512].T + b1     # [2000, 512]   (node table)
    B = z_reaction @ W1[:, 512:]            # [10000, 512]  (node table)
    out[e] = w2 . relu(A[row[e]] + B[col[e]]) + b2

Sharding: edges are bucketed by col//1250, so core i only needs the
z_reaction rows [1250*i, 1250*(i+1)) — this cuts replicated input staging
~4x vs full-table replication (z_reaction is the dominant input tensor).
z_drug (2 MB) is replicated since any drug can pair with any reaction.

Device schedule (SPMD; core i owns the edges whose col lands in its shard):
  Phase 0: broadcast b1 across partitions via a 1-contraction PE matmul;
           replicate the [16, *] host index array to 128 partitions.
  Phase 1: precompute T = [A (2048 rows); B_shard (1280 rows)] on the PE,
           write fp16 row-major to DRAM scratch.
  Phase 2: per 2048-edge tile: two transposed dma_gathers from the
           combined table T (row idx for A, 2048+local_col for B; the DMA
           descriptor ring caps one transpose gather at 2048 indices),
           DVE add, relu (split DVE/ACT), PE matvec with w2
           (PSUM accumulate), +b2, DMA out.

Host side: bucket/permute edges, build wrapped int16 indices, scatter the
per-core outputs back to the global edge order.
"""

import numpy as np

H = 512
N_DRUG, N_REACTION, N_EDGES = 2000, 10000, 400000
N_CORES = 8
B_SH = N_REACTION // N_CORES        # 1250 real reaction rows per core
B_SH_PAD = 1280                     # padded to a multiple of 128
A_ROWS = 2048                       # drug table rows padded
T_ROWS = A_ROWS + B_SH_PAD          # 3328 combined-table rows
ET = 2048                           # edges per tile
NT = 26                             # tiles per core (capacity 53248 >= ~50k+15sigma)
E_PAD = NT * ET
IDXC = ET // 16                     # 128 idx columns per tile (2048 idxs wrapped in 16)
KC = H // 128                       # 4 contraction chunks of 128

_CACHE = {}


def _build_nc():
    import concourse.bacc as bacc
    import concourse.mybir as mybir
    import concourse.tile as tile
    from concourse import library_config
    from concourse.bass import ts

    dt = mybir.dt
    nc = bacc.Bacc(None, target_bir_lowering=False)

    zdT = nc.dram_tensor("zdT", [H, A_ROWS], dt.float16, kind="ExternalInput")
    zrTs = nc.dram_tensor("zrTs", [H, B_SH_PAD], dt.float16, kind="ExternalInput")
    w1dT = nc.dram_tensor("w1dT", [H, H], dt.float16, kind="ExternalInput")
    w1rT = nc.dram_tensor("w1rT", [H, H], dt.float16, kind="ExternalInput")
    b1v = nc.dram_tensor("b1v", [1, H], dt.float32, kind="ExternalInput")
    w2T = nc.dram_tensor("w2T", [128, KC * 32], dt.float16, kind="ExternalInput")
    b2v = nc.dram_tensor("b2v", [128, 1], dt.float32, kind="ExternalInput")
    rowi16 = nc.dram_tensor("rowi16", [16, NT * IDXC], dt.int16, kind="ExternalInput")
    coli16 = nc.dram_tensor("coli16", [16, NT * IDXC], dt.int16, kind="ExternalInput")
    out = nc.dram_tensor("out", [E_PAD], dt.float32, kind="ExternalOutput")

    with tile.TileContext(nc) as tc:
        with (
            tc.tile_pool(name="const", bufs=1) as cpool,
            tc.tile_pool(name="z", bufs=2) as zpool,
            tc.tile_pool(name="o1", bufs=3) as opool,
            tc.tile_pool(name="g", bufs=3) as gpool,
            tc.tile_pool(name="tt", bufs=2) as tpool,
            tc.tile_pool(name="fin", bufs=2) as fpool,
            tc.tile_pool(name="ps1", bufs=3, space="PSUM") as ps1,
            tc.tile_pool(name="psb", bufs=1, space="PSUM") as psb,
            tc.tile_pool(name="ps2", bufs=4, space="PSUM") as ps2,
            tc.tile_pool(name="dram", bufs=1, space="DRAM") as dpool,
        ):
            # dma_gather (DMAGatherAnt) lives in the 'mlp' GPSIMD library
            nc.gpsimd.load_library(library_config.mlp)

            # ---- constant / index preload ----
            w1d_sb = cpool.tile([128, KC, H], dt.float16)
            nc.sync.dma_start(
                out=w1d_sb[:], in_=w1dT[:, :].rearrange("(c p) o -> p c o", p=128)
            )
            w1r_sb = cpool.tile([128, KC, H], dt.float16)
            nc.sync.dma_start(
                out=w1r_sb[:], in_=w1rT[:, :].rearrange("(c p) o -> p c o", p=128)
            )
            w2_sb = cpool.tile([128, KC, 32], dt.float16)
            nc.sync.dma_start(
                out=w2_sb[:], in_=w2T[:, :].rearrange("p (c m) -> p c m", m=32)
            )
            b2_sb = cpool.tile([128, 1], dt.float32)
            nc.sync.dma_start(out=b2_sb[:], in_=b2v[:, :])

            # b1 broadcast: ones[1,128] (x) b1row[1,512] -> [128,512]
            b1_row = cpool.tile([1, H], dt.float32)
            nc.sync.dma_start(out=b1_row[:], in_=b1v[:, :])
            ones_sb = cpool.tile([1, 128], dt.float32)
            nc.vector.memset(ones_sb[:], 1.0)
            b1_ps = psb.tile([128, H], dt.float32, tag="b1ps")
            nc.tensor.matmul(
                out=b1_ps[:], lhsT=ones_sb[:, :], rhs=b1_row[:, :],
                start=True, stop=True,
            )
            b1_sb = cpool.tile([128, H], dt.float32)
            nc.scalar.copy(out=b1_sb[:], in_=b1_ps[:])

            # index replicate 16 -> 128 partitions (8 DMA reads of same DRAM)
            row_sb = cpool.tile([128, NT * IDXC], dt.int16)
            col_sb = cpool.tile([128, NT * IDXC], dt.int16)
            for k in range(8):
                nc.sync.dma_start(
                    out=row_sb[16 * k : 16 * (k + 1), :], in_=rowi16[:, :]
                )
                nc.sync.dma_start(
                    out=col_sb[16 * k : 16 * (k + 1), :], in_=coli16[:, :]
                )

            T_t = dpool.tile([T_ROWS, H], dt.float16, tag="T")

            # ---- phase 1: T[0:2048] = zd@W1d.T + b1 ; T[2048:] = zr_sh@W1r.T ----
            def precompute(zT_handle, w1_sb, width, zblk, row_base, add_b1):
                z_ap = zT_handle[:, :].rearrange(
                    "(c p) (b n) -> b p c n", p=128, n=zblk
                )
                for b in range(width // zblk):
                    zt = zpool.tile([128, KC, zblk], dt.float16, tag="zt")
                    nc.sync.dma_start(out=zt[:], in_=z_ap[b])
                    for nt_ in range(zblk // 128):
                        psum = ps1.tile([128, H], dt.float32, tag="ps1")
                        for c in range(KC):
                            nc.tensor.matmul(
                                out=psum[:],
                                lhsT=zt[:, c, ts(nt_, 128)],
                                rhs=w1_sb[:, c, :],
                                start=(c == 0),
                                stop=(c == KC - 1),
                            )
                        osb = opool.tile([128, H], dt.float16, tag="osb")
                        if add_b1:
                            nc.vector.tensor_add(out=osb[:], in0=psum[:], in1=b1_sb[:])
                        else:
                            nc.scalar.copy(out=osb[:], in_=psum[:])
                        r0 = row_base + b * zblk + nt_ * 128
                        nc.sync.dma_start(out=T_t[r0 : r0 + 128, :], in_=osb[:])

            precompute(zdT, w1d_sb, A_ROWS, 1024, 0, add_b1=True)
            precompute(zrTs, w1r_sb, B_SH_PAD, B_SH_PAD, A_ROWS, add_b1=False)

            # ---- phase 2: merged gather + add + relu + w2 matvec ----
            out_ap = out[:].rearrange("(t g n) -> t g n", g=4, n=512)
            for t in range(NT):
                ag = gpool.tile([128, KC, ET], dt.float16, tag="ag")
                nc.gpsimd.dma_gather(
                    out_ap=ag[:],
                    in_ap=T_t[:, :],
                    idxs_ap=row_sb[:, ts(t, IDXC)],
                    num_idxs=ET,
                    num_idxs_reg=ET,
                    elem_size=H,
                    transpose=True,
                    single_packet=False,
                )
                bg = gpool.tile([128, KC, ET], dt.float16, tag="bg")
                nc.gpsimd.dma_gather(
                    out_ap=bg[:],
                    in_ap=T_t[:, :],
                    idxs_ap=col_sb[:, ts(t, IDXC)],
                    num_idxs=ET,
                    num_idxs_reg=ET,
                    elem_size=H,
                    transpose=True,
                    single_packet=False,
                )
                tt = tpool.tile([128, KC, ET], dt.float16, tag="tt")
                nc.vector.tensor_add(out=tt[:], in0=ag[:], in1=bg[:])
                if t % 3 == 0:
                    nc.vector.tensor_scalar_max(out=tt[:], in0=tt[:], scalar1=0.0)
                else:
                    nc.scalar.activation(
                        out=tt[:], in_=tt[:], func=mybir.ActivationFunctionType.Relu
                    )
                psum = ps2.tile([128, 512], dt.float32, tag="ps2")
                for g in range(4):
                    # w2 chunk replicated over 32 PE columns: group g fills
                    # psum partitions [g*32, (g+1)*32) with identical rows, so
                    # the whole bank is written (no uninitialized reads) and
                    # the finisher is a single full-tile op. PE output base
                    # partition must be 32-aligned; explicit tile_position
                    # because base_partition() rejects 96.
                    for c in range(KC):
                        nc.tensor.matmul(
                            out=psum[g * 32 : (g + 1) * 32, :],
                            lhsT=w2_sb[:, c, :],
                            rhs=tt[:, c, ts(g, 512)],
                            start=(c == 0),
                            stop=(c == KC - 1),
                            tile_position=(0, g * 32),
                        )
                fsb = fpool.tile([128, 512], dt.float32, tag="fout")
                nc.vector.tensor_scalar_add(
                    out=fsb[:], in0=psum[:, :], scalar1=b2_sb[:, :]
                )
                nc.sync.dma_start(out=out_ap[t], in_=fsb[::32, :])
    nc.compile()
    return nc


def _wrap_idx(a):
    """[NT*ET] int -> [16, NT*IDXC] int16 in dma_gather's wrapped layout.

    Within tile t, index j (0..2047) sits at partition j%16, free column
    t*IDXC + j//16. Replication to 128 partitions happens on device.
    """
    m = a.reshape(NT, IDXC, 16)              # [t, j//16, j%16]
    w = m.transpose(2, 0, 1).reshape(16, NT * IDXC)
    return np.ascontiguousarray(w, dtype=np.int16)


def get_nc():
    if "nc" not in _CACHE:
        _CACHE["nc"] = _build_nc()
    return _CACHE["nc"]


def make_in_maps(z_drug, z_reaction, row, col, W1, b1, W2, b2):
    f16 = np.float16
    zdT = np.zeros((H, A_ROWS), f16)
    zdT[:, :N_DRUG] = np.asarray(z_drug, np.float32).T.astype(f16)
    zrT = np.asarray(z_reaction, np.float32).T.astype(f16)  # [H, 10000]
    W1 = np.asarray(W1, np.float32)
    w1dT = np.ascontiguousarray(W1[:, :H].T).astype(f16)
    w1rT = np.ascontiguousarray(W1[:, H:].T).astype(f16)
    b1v = np.asarray(b1, np.float32).reshape(1, H)
    # w2T[p, c*32 + m] = W2[0, c*128 + p]  (chunk value replicated over 32 cols)
    w2c = np.asarray(W2, np.float32).reshape(KC, 128).T.astype(f16)  # [128, KC]
    w2T = np.ascontiguousarray(
        np.repeat(w2c[:, :, None], 32, axis=2).reshape(128, KC * 32)
    )
    b2v = np.full((128, 1), float(np.asarray(b2).reshape(-1)[0]), np.float32)
    row = np.asarray(row).astype(np.int64)
    col = np.asarray(col).astype(np.int64)

    # bucket edges by reaction shard
    bucket = col // B_SH
    order = np.argsort(bucket, kind="stable")
    counts = np.bincount(bucket, minlength=N_CORES)
    assert counts.max() <= E_PAD, f"bucket overflow: {counts.max()} > {E_PAD}"
    bounds = np.concatenate([[0], np.cumsum(counts)])

    in_maps = []
    for ci in range(N_CORES):
        sel = order[bounds[ci] : bounds[ci + 1]]
        n = len(sel)
        zs = np.zeros((H, B_SH_PAD), f16)
        zs[:, :B_SH] = zrT[:, ci * B_SH : (ci + 1) * B_SH]
        r = np.zeros(E_PAD, np.int64)
        r[:n] = row[sel]
        c = np.full(E_PAD, A_ROWS, np.int64)
        c[:n] = col[sel] - ci * B_SH + A_ROWS
        in_maps.append(
            {
                "zdT": zdT,
                "zrTs": zs,
                "w1dT": w1dT,
                "w1rT": w1rT,
                "b1v": b1v,
                "w2T": w2T,
                "b2v": b2v,
                "rowi16": _wrap_idx(r),
                "coli16": _wrap_idx(c),
            }
        )
    return in_maps, order, counts


def kernel(z_drug, z_reaction, row, col, W1, b1, W2, b2):
    from concourse.bass_utils import run_bass_kernel_spmd

    nc = get_nc()
    in_maps, order, counts = make_in_maps(
        z_drug, z_reaction, row, col, W1, b1, W2, b2
    )
    res = run_bass_kernel_spmd(nc, in_maps, core_ids=list(range(N_CORES)))
    out_full = np.empty(N_EDGES, np.float32)
    for ci in range(N_CORES):
        lo = 0 if ci == 0 else int(np.cumsum(counts)[ci - 1])
        out_full[order[lo : lo + counts[ci]]] = res.results[ci]["out"][: counts[ci]]
    return out_full
